# revision 16
# baseline (speedup 1.0000x reference)
"""Trainium2 Bass kernel for the LSTM caption decoder (nn_Decoder_62483184222858).

Math (per reference):
    emb = embed_W[captions]                      # [B, T, E]
    h0 = feature, c0 = 0
    for t in 0..T-2:
        gates = x_t @ W_ih.T + h @ W_hh.T + (b_ih+b_hh)   # [B, 4H] order i,f,g,o
        i, f, o = sigmoid(...); g = tanh(g)
        c = f*c + i*g
        h = o*tanh(c) + feature                   # emitted output AND carried state
    logits = outs @ lin_W.T + lin_b               # [B, T-1, V]

Strategy: data-parallel over 8 NeuronCores (64 batch rows each).

The recurrent matmul is computed TRANSPOSED (gatesT[4H, B] = W_hh @ h.T)
with W_hh tiles as the 128x128 stationary operand and hT chunks [128, 64]
as the moving operand: the cost-model price of a matmul is its output
free-size, so this halves the gate cost vs. streaming W_hh columns.
All elementwise state (c, h, activations) lives in chunk-major layout
[128 part, (chunk, batch)] so ops batch into full [128, 512] instructions,
and h IS the next step's moving operand (no per-step PE transposes).

Per step:
  - xp[64, 4096] gathered from the precomputed token table (phase A:
    tokp[v] = embed_W[v] @ W_ih.T + bias, bf16), then XBAR-DMA-transposed
    into chunk-major xpT [128, (gj, b)].
  - gate PSUM init via identity-matmul of xpT (start=True), then 64
    accumulating W-MMs per gate streaming hT chunks.
  - ACT sigmoid/tanh straight from PSUM; DVE c/h chain in [128, 512] ops;
    o-gate tail split in halves so h lands early.
  - logits computed per step-pair from the H4 ring (M=128), issued as PE
    gap filler at the top of each step.
"""

import sys

if "/opt/trn_rl_repo" not in sys.path:
    sys.path.insert(0, "/opt/trn_rl_repo")

import numpy as np
import ml_dtypes

import concourse.bass as bass
import concourse.mybir as mybir
import concourse.tile as tile
from concourse import bacc
from concourse.bass_utils import run_bass_kernel_spmd
from concourse.masks import make_identity

F32 = mybir.dt.float32
BF16 = mybir.dt.bfloat16
I32 = mybir.dt.int32
AF = mybir.ActivationFunctionType

EMBED, HIDDEN, VOCAB = 512, 1024, 1004
B, T = 512, 65
NCORES = 8
BL = B // NCORES          # 64 batch rows per core
TS = T - 1                # 64 time steps
G4 = 4 * HIDDEN           # 4096 gate width
KK_H = HIDDEN // 128      # 8 contraction chunks over hidden
KK_E = EMBED // 128       # 4 contraction chunks over embed
NVT = (VOCAB + 127) // 128  # 8 vocab tiles (last is 108 rows)
NGJ = G4 // 128           # 32 gate-channel tiles
HB = KK_H * BL            # 512: one h/c tile's free width (chunk-major)

# blob_a (bf16) layout: embWT | WihT  (k-chunk-major per-partition free dim)
A_EMB = 0
A_WIH = A_EMB + KK_E * VOCAB            # 4016
A_END = A_WIH + KK_E * G4               # 20400

# gate stream order (torch gate indices): g, i, f, o
GSTREAM = (2, 0, 1, 3)


def build_program(steps=TS):
    nc = bacc.Bacc("TRN2", target_bir_lowering=False, debug=False)

    blob_a = nc.dram_tensor("blob_a", [128, A_END], BF16, kind="ExternalInput")
    biasg = nc.dram_tensor("biasg", [1, G4], BF16, kind="ExternalInput")
    whhT = nc.dram_tensor("whhT", [128, NGJ * KK_H * 128], BF16,
                          kind="ExternalInput")
    featT = nc.dram_tensor("featT", [128, HB], BF16, kind="ExternalInput")
    caps = nc.dram_tensor("caps", [BL, TS], I32, kind="ExternalInput")
    linWT = nc.dram_tensor("linWT", [128, KK_H * VOCAB], BF16,
                           kind="ExternalInput")
    linb = nc.dram_tensor("linb", [1, VOCAB], F32, kind="ExternalInput")
    out = nc.dram_tensor("out", [BL, TS, VOCAB], F32, kind="ExternalOutput")

    tokp = nc.dram_tensor("tokp", [VOCAB, G4], BF16, kind="Internal")

    with tile.TileContext(nc) as tc:
        _body(nc, tc, steps,
              blob_a.ap(), biasg.ap(), whhT.ap(), featT.ap(), caps.ap(),
              linWT.ap(), linb.ap(), out.ap(), tokp.ap())
    nc.compile()
    return nc


def _body(nc, tc, steps, blob_a, biasg, whhT, featT, caps, linWT, linb, out,
          tokp):
    with tc.tile_pool(name="pg", bufs=1) as pg:
        ident = pg.tile([128, 128], BF16, tag="ident")
        make_identity(nc, ident[:])

        # ================= phase A: token table =========================
        with (
            tc.tile_pool(name="pa", bufs=1) as pa,
            tc.tile_pool(name="pap", bufs=1, space="PSUM") as pap,
        ):
            ba = pa.tile([128, A_END], BF16, tag="blob_a")
            # per-k-chunk loads so the first matmuls start early
            nc.sync.dma_start(ba[:, 0:A_WIH], blob_a[:, 0:A_WIH])
            for k in range(KK_E):
                c0, c1 = A_WIH + k * G4, A_WIH + (k + 1) * G4
                nc.sync.dma_start(ba[:, c0:c1], blob_a[:, c0:c1])
            embWT_sb = ba[:, A_EMB:A_EMB + KK_E * VOCAB]
            WihT_sb = ba[:, A_WIH:A_WIH + KK_E * G4]
            bias_sb = pa.tile([128, G4], BF16, tag="bias")
            nc.sync.dma_start(bias_sb[:], biasg.to_broadcast((128, G4)))

            # loop-phase constants: whh interleaved below; rest at loop start
            whh_sb = pg.tile([128, NGJ * KK_H * 128], BF16, tag="whh")
            featT_sb = pg.tile([128, HB], BF16, tag="featT")
            caps_sb = pg.tile([BL, TS], I32, tag="caps")
            linWT_sb = pg.tile([128, KK_H * VOCAB], BF16, tag="linWT")
            lb_sb = pg.tile([128, VOCAB], F32, tag="lb")
            # h ring: [p, k*(4*BL) + slot*BL + b] so a step-pair's k-chunk
            # slice is contiguous (matmul lhsT needs a single free dim)
            H4 = pg.tile([128, 4 * HB], BF16, tag="H4")

            # whh load order follows the gate stream order, split in 8 so
            # small tokp writes interleave on the DMA rings
            whh_chunks = []
            for gi in GSTREAM:
                c0 = gi * 8 * KK_H * 128
                cm = c0 + 4 * KK_H * 128
                c1 = c0 + 8 * KK_H * 128
                whh_chunks.append((c0, cm))
                whh_chunks.append((cm, c1))

            for m in range(NVT):
                mrows = min(128, VOCAB - 128 * m)
                for nb in range(G4 // 512):
                    ps = pap.tile([128, 512], F32, tag="tp_ps", bufs=8)
                    for k in range(KK_E):
                        nc.tensor.matmul(
                            ps[:mrows],
                            lhsT=embWT_sb[:, k * VOCAB + 128 * m:
                                          k * VOCAB + 128 * m + mrows],
                            rhs=WihT_sb[:, k * G4 + 512 * nb:
                                        k * G4 + 512 * (nb + 1)],
                            start=(k == 0), stop=(k == KK_E - 1),
                        )
                    sb = pa.tile([128, 512], BF16, tag="tp_sb", bufs=24)
                    nc.vector.tensor_add(
                        sb[:mrows], ps[:mrows],
                        bias_sb[:mrows, 512 * nb:512 * (nb + 1)])
                    nc.sync.dma_start(
                        tokp[128 * m:128 * m + mrows, 512 * nb:512 * (nb + 1)],
                        sb[:mrows])
                c0, c1 = whh_chunks[m]
                nc.sync.dma_start(whh_sb[:, c0:c1], whhT[:, c0:c1])
                if m == NVT - 1:
                    nc.sync.dma_start(featT_sb[:], featT)
                    nc.sync.dma_start(caps_sb[:], caps)

        # ================= recurrence + in-loop logits ===================
        with (
            tc.tile_pool(name="pb", bufs=1) as pb,
            tc.tile_pool(name="pbp", bufs=1, space="PSUM") as pbp,
        ):
            H4v = H4[:].rearrange("p (k s b) -> p k s b", k=KK_H, s=4)

            # deferred constant loads (needed from t>=2 only)
            nc.sync.dma_start(linWT_sb[:], linWT)
            nc.sync.dma_start(lb_sb[:], linb.to_broadcast((128, VOCAB)))

            c_cur = pb.tile([128, HB], F32, tag="c", bufs=2, name="c_init")
            nc.vector.memset(c_cur[:], 0.0)

            xp_t = {}
            xpT_t = {}

            def gather(t):
                xp_t[t] = pb.tile([BL, G4], BF16, tag="xp", bufs=3,
                                  name=f"xp_{t}")
                nc.gpsimd.indirect_dma_start(
                    out=xp_t[t][:], out_offset=None, in_=tokp,
                    in_offset=bass.IndirectOffsetOnAxis(
                        ap=caps_sb[:, t:t + 1], axis=0),
                )

            def transp(t):
                xpT_t[t] = pb.tile([128, NGJ * BL], BF16, tag="xpT", bufs=2,
                                   name=f"xpT_{t}")
                nc.sync.dma_start(
                    xpT_t[t][:].rearrange("p (j b) -> p j b", j=NGJ),
                    xp_t[t][:], transpose=True)
                del xp_t[t]

            gather(0)
            gather(1)
            transp(0)

            lp_cur = [None]

            def logits_half(p, nh):
                # logits for out steps 2p, 2p+1 from h_{2p+1}, h_{2p+2}
                # (H4 slots 2p%4, 2p%4+1 -- never wraps since 2p%4 in {0,2})
                if nh == 0:
                    lp_cur[0] = pbp.tile([128, 1024], F32, tag="lp", bufs=1,
                                         name=f"lp_{p}")
                lp = lp_cur[0]
                n0, n1 = (0, 512) if nh == 0 else (512, VOCAB)
                s0 = (2 * p) % 4
                for k in range(KK_H):
                    nc.tensor.matmul(
                        lp[:, n0:n1],
                        lhsT=H4[:, k * 4 * BL + s0 * BL:
                                k * 4 * BL + (s0 + 2) * BL],
                        rhs=linWT_sb[:, k * VOCAB + n0:k * VOCAB + n1],
                        start=(k == 0), stop=(k == KK_H - 1))

            def logits_tail(p):
                ls = pb.tile([128, VOCAB], F32, tag="ls", bufs=2,
                             name=f"ls_{p}")
                nc.vector.tensor_add(ls[:], lp_cur[0][:, 0:VOCAB], lb_sb[:])
                nc.sync.dma_start(out[:, 2 * p, :], ls[0:BL])
                nc.sync.dma_start(out[:, 2 * p + 1, :], ls[BL:128])

            for t in range(steps):
                if t + 2 < steps:
                    gather(t + 2)
                if t + 1 < steps:
                    transp(t + 1)

                if t == 0:
                    def hs(k):
                        return featT_sb[:, k * BL:(k + 1) * BL]
                else:
                    sp = (t - 1) % 4

                    def hs(k, sp=sp):
                        return H4[:, k * 4 * BL + sp * BL:
                                  k * 4 * BL + (sp + 1) * BL]

                # PE gap filler: logits half-pair (reads h from >=2 steps
                # back, so it runs while the current h-tail is in flight)
                if t >= 3 and t % 2 == 1:
                    logits_half((t - 3) // 2, 0)
                elif t >= 4 and t % 2 == 0:
                    logits_half((t - 4) // 2, 1)

                # recurrent gate matmuls: W_hh tiles stationary, hT moving.
                # each j-tile is its own accumulation group within the bank,
                # so the zero-region group checker must be skipped.
                gps = {}
                for gi in GSTREAM:
                    gps[gi] = pbp.tile([128, 512], F32, tag="gps", bufs=4,
                                       name=f"g{gi}_{t}")
                    base = gi * 8 * KK_H * 128
                    for j in range(8):
                        oap = gps[gi][:, j * BL:(j + 1) * BL]
                        for k in range(KK_H):
                            nc.tensor.matmul(
                                oap,
                                lhsT=whh_sb[:, base + (j * KK_H + k) * 128:
                                            base + (j * KK_H + k + 1) * 128],
                                rhs=hs(k),
                                start=(k == 0), stop=(k == KK_H - 1),
                                skip_group_check=True,
                            )

                # add the token projection (DVE), then activations
                xpT = xpT_t.pop(t)

                gg = pb.tile([128, HB], F32, tag="gg", name=f"gg_{t}")
                nc.vector.tensor_add(gg[:], gps[2][:], xpT[:, 1024:1536])
                gi_ = pb.tile([128, HB], F32, tag="gi", name=f"gi_{t}")
                nc.vector.tensor_add(gi_[:], gps[0][:], xpT[:, 0:512])
                gf = pb.tile([128, HB], F32, tag="gf", name=f"gf_{t}")
                nc.vector.tensor_add(gf[:], gps[1][:], xpT[:, 512:1024])

                ag = pb.tile([128, HB], BF16, tag="ag", name=f"ag_{t}")
                nc.scalar.activation(ag[:], gg[:], AF.Tanh)
                ai = pb.tile([128, HB], BF16, tag="ai", name=f"ai_{t}")
                nc.scalar.activation(ai[:], gi_[:], AF.Sigmoid)
                af = pb.tile([128, HB], BF16, tag="af", name=f"af_{t}")
                nc.scalar.activation(af[:], gf[:], AF.Sigmoid)

                t2 = pb.tile([128, HB], F32, tag="t2", name=f"t2_{t}")
                nc.vector.tensor_mul(t2[:], ai[:], ag[:])
                t1 = pb.tile([128, HB], F32, tag="t1", name=f"t1_{t}")
                nc.vector.tensor_mul(t1[:], af[:], c_cur[:])
                c_new = pb.tile([128, HB], F32, tag="c", bufs=2,
                                name=f"c_{t}")
                nc.vector.tensor_add(c_new[:], t1[:], t2[:])
                tcb = pb.tile([128, HB], BF16, tag="tc", name=f"tc_{t}")
                nc.scalar.activation(tcb[:], c_new[:], AF.Tanh)

                # o-gate tail in halves so the first hT chunks land early
                sw = t % 4
                for hh in range(2):
                    sl = slice(hh * 256, (hh + 1) * 256)
                    go = pb.tile([128, 256], F32, tag="go", bufs=2,
                                 name=f"go{hh}_{t}")
                    nc.vector.tensor_add(go[:], gps[3][:, sl],
                                         xpT[:, 1536 + sl.start:
                                             1536 + sl.stop])
                    ao = pb.tile([128, 256], BF16, tag="ao", bufs=2,
                                 name=f"ao{hh}_{t}")
                    nc.scalar.activation(ao[:], go[:], AF.Sigmoid)
                    t3 = pb.tile([128, 256], BF16, tag="t3", bufs=2,
                                 name=f"t3{hh}_{t}")
                    nc.vector.tensor_mul(t3[:], ao[:], tcb[:, sl])
                    nc.vector.tensor_add(
                        H4v[:, hh * 4:(hh + 1) * 4, sw, :],
                        t3[:], featT_sb[:, sl])
                c_cur = c_new

                # logits epilogue late in the step (keeps DVE queue clear)
                if t >= 4 and t % 2 == 0:
                    logits_tail((t - 4) // 2)

            # trailing logits
            logits_half((steps - 4) // 2, 1)      # p=30 half B
            logits_tail((steps - 4) // 2)
            logits_half((steps - 2) // 2, 0)      # p=31
            logits_half((steps - 2) // 2, 1)
            logits_tail((steps - 2) // 2)

# ---------------------------------------------------------------------------
# host glue
# ---------------------------------------------------------------------------

_CACHE = {}


def _get_program(steps=TS):
    if steps not in _CACHE:
        _CACHE[steps] = build_program(steps)
    return _CACHE[steps]


def chunkT(w, kk):
    # [R, C] -> [128, kk*C]: [p, k*C + c] = w.T[k*128+p, c] = w[c, k*128+p]
    f32 = np.float32
    wt = np.ascontiguousarray(w.T.astype(f32))
    r = wt.reshape(kk, 128, w.shape[0])
    return np.ascontiguousarray(r.transpose(1, 0, 2).reshape(128, -1))


def make_in_maps(feature, captions, embed_W, W_ih, W_hh, b_ih, b_hh,
                 lin_W, lin_b):
    f32 = np.float32
    bf16 = ml_dtypes.bfloat16

    embWT_p = chunkT(embed_W, KK_E)          # [128, 4*1004]
    WihT_p = chunkT(W_ih, KK_E)              # [128, 4*4096]
    blob_a = np.concatenate([embWT_p, WihT_p], axis=1).astype(bf16)

    # whhT: [p, (GJ*8 + k)*128 + m] = W_hh[GJ*128 + m, k*128 + p]
    arr = W_hh.astype(f32).reshape(NGJ, 128, KK_H, 128)   # [GJ, m, k, p]
    whhT_p = np.ascontiguousarray(
        arr.transpose(3, 0, 2, 1).reshape(128, NGJ * KK_H * 128)).astype(bf16)

    linWT_p = chunkT(lin_W, KK_H).astype(bf16)   # [128, 8*1004]

    shared = {
        "blob_a": np.ascontiguousarray(blob_a),
        "biasg": (b_ih + b_hh).astype(f32).reshape(1, G4).astype(bf16),
        "whhT": whhT_p,
        "linWT": np.ascontiguousarray(linWT_p),
        "linb": lin_b.astype(f32).reshape(1, VOCAB),
    }
    in_maps = []
    for i in range(NCORES):
        sl = slice(i * BL, (i + 1) * BL)
        fl = np.ascontiguousarray(feature[sl].astype(f32))
        featT_p = np.ascontiguousarray(
            fl.T.reshape(KK_H, 128, BL).transpose(1, 0, 2).reshape(128, HB))
        m = dict(shared)
        m["featT"] = featT_p.astype(bf16)
        m["caps"] = np.ascontiguousarray(captions[sl, :TS].astype(np.int32))
        in_maps.append(m)
    return in_maps


def kernel(feature, captions, lengths=None, embed_W=None, W_ih=None,
           W_hh=None, b_ih=None, b_hh=None, lin_W=None, lin_b=None,
           trace=False):
    feature = np.asarray(feature)
    captions = np.asarray(captions)
    nc = _get_program()
    in_maps = make_in_maps(
        feature, captions, np.asarray(embed_W), np.asarray(W_ih),
        np.asarray(W_hh), np.asarray(b_ih), np.asarray(b_hh),
        np.asarray(lin_W), np.asarray(lin_b))
    res = run_bass_kernel_spmd(nc, in_maps, list(range(NCORES)), trace=trace)
    outp = np.concatenate([res.results[i]["out"] for i in range(NCORES)], axis=0)
    if trace:
        kernel.last_exec_time_ns = res.exec_time_ns
        kernel.last_results = res
    return outp


# revision 18
# speedup vs baseline: 1.1988x; 1.1988x over previous
"""Trainium2 Bass kernel for the LSTM caption decoder (nn_Decoder_62483184222858).

Math (per reference):
    emb = embed_W[captions]                      # [B, T, E]
    h0 = feature, c0 = 0
    for t in 0..T-2:
        gates = x_t @ W_ih.T + h @ W_hh.T + (b_ih+b_hh)   # [B, 4H] order i,f,g,o
        i, f, o = sigmoid(...); g = tanh(g)
        c = f*c + i*g
        h = o*tanh(c) + feature                   # emitted output AND carried state
    logits = outs @ lin_W.T + lin_b               # [B, T-1, V]

Strategy: data-parallel over 8 NeuronCores (64 batch rows each).

The recurrent matmul is computed TRANSPOSED (gatesT[4H, B] = W_hh @ h.T)
with W_hh tiles as the 128x128 stationary operand and hT chunks [128, 64]
as the moving operand: the cost-model price of a matmul is its output
free-size, so this halves the gate cost vs. streaming W_hh columns.
All elementwise state (c, h, activations) lives in chunk-major layout
[128 part, (chunk, batch)] so ops batch into full [128, 512] instructions,
and h IS the next step's moving operand (no per-step PE transposes).

Per step:
  - xp[64, 4096] gathered from the precomputed token table (phase A:
    tokp[v] = embed_W[v] @ W_ih.T + bias, bf16), then XBAR-DMA-transposed
    into chunk-major xpT [128, (gj, b)].
  - gate PSUM init via identity-matmul of xpT (start=True), then 64
    accumulating W-MMs per gate streaming hT chunks.
  - ACT sigmoid/tanh straight from PSUM; DVE c/h chain in [128, 512] ops;
    o-gate tail split in halves so h lands early.
  - logits computed per step-pair from the H4 ring (M=128), issued as PE
    gap filler at the top of each step.
"""

import sys

if "/opt/trn_rl_repo" not in sys.path:
    sys.path.insert(0, "/opt/trn_rl_repo")

import numpy as np
import ml_dtypes

import concourse.bass as bass
import concourse.mybir as mybir
import concourse.tile as tile
from concourse import bacc
from concourse.bass_utils import run_bass_kernel_spmd
from concourse.masks import make_identity

F32 = mybir.dt.float32
BF16 = mybir.dt.bfloat16
I32 = mybir.dt.int32
AF = mybir.ActivationFunctionType

EMBED, HIDDEN, VOCAB = 512, 1024, 1004
B, T = 512, 65
NCORES = 8
BL = B // NCORES          # 64 batch rows per core
TS = T - 1                # 64 time steps
G4 = 4 * HIDDEN           # 4096 gate width
KK_H = HIDDEN // 128      # 8 contraction chunks over hidden
KK_E = EMBED // 128       # 4 contraction chunks over embed
NVT = (VOCAB + 127) // 128  # 8 vocab tiles (last is 108 rows)
NGJ = G4 // 128           # 32 gate-channel tiles
HB = KK_H * BL            # 512: one h/c tile's free width (chunk-major)

# blob_a (bf16) layout: embWT | WihT  (k-chunk-major per-partition free dim)
A_EMB = 0
A_WIH = A_EMB + KK_E * VOCAB            # 4016
A_END = A_WIH + KK_E * G4               # 20400

# gate stream order (torch gate indices): g, i, f, o
GSTREAM = (2, 0, 1, 3)


def build_program(steps=TS):
    nc = bacc.Bacc("TRN2", target_bir_lowering=False, debug=False)

    blob_a = nc.dram_tensor("blob_a", [128, A_END], BF16, kind="ExternalInput")
    biasg = nc.dram_tensor("biasg", [1, G4], BF16, kind="ExternalInput")
    whhT = nc.dram_tensor("whhT", [128, NGJ * KK_H * 128], BF16,
                          kind="ExternalInput")
    featT = nc.dram_tensor("featT", [128, HB], BF16, kind="ExternalInput")
    caps = nc.dram_tensor("caps", [BL, TS], I32, kind="ExternalInput")
    linWT = nc.dram_tensor("linWT", [128, KK_H * VOCAB], BF16,
                           kind="ExternalInput")
    linb = nc.dram_tensor("linb", [1, VOCAB], F32, kind="ExternalInput")
    out = nc.dram_tensor("out", [BL, TS, VOCAB], F32, kind="ExternalOutput")

    tokp = nc.dram_tensor("tokp", [VOCAB, G4], BF16, kind="Internal")

    with tile.TileContext(nc) as tc:
        _body(nc, tc, steps,
              blob_a.ap(), biasg.ap(), whhT.ap(), featT.ap(), caps.ap(),
              linWT.ap(), linb.ap(), out.ap(), tokp.ap())
    nc.compile()
    return nc


def _body(nc, tc, steps, blob_a, biasg, whhT, featT, caps, linWT, linb, out,
          tokp):
    with tc.tile_pool(name="pg", bufs=1) as pg:
        ident = pg.tile([128, 128], BF16, tag="ident")
        make_identity(nc, ident[:])

        # ================= phase A: token table =========================
        with (
            tc.tile_pool(name="pa", bufs=1) as pa,
            tc.tile_pool(name="pap", bufs=1, space="PSUM") as pap,
        ):
            ba = pa.tile([128, A_END], BF16, tag="blob_a")
            # per-k-chunk loads so the first matmuls start early
            nc.sync.dma_start(ba[:, 0:A_WIH], blob_a[:, 0:A_WIH])
            for k in range(KK_E):
                c0, c1 = A_WIH + k * G4, A_WIH + (k + 1) * G4
                nc.sync.dma_start(ba[:, c0:c1], blob_a[:, c0:c1])
            embWT_sb = ba[:, A_EMB:A_EMB + KK_E * VOCAB]
            WihT_sb = ba[:, A_WIH:A_WIH + KK_E * G4]
            bias_sb = pa.tile([128, G4], BF16, tag="bias")
            nc.sync.dma_start(bias_sb[:], biasg.to_broadcast((128, G4)))

            # loop-phase constants: whh interleaved below; rest at loop start
            whh_sb = pg.tile([128, NGJ * KK_H * 128], BF16, tag="whh")
            featT_sb = pg.tile([128, HB], BF16, tag="featT")
            caps_sb = pg.tile([BL, TS], I32, tag="caps")
            linWT_sb = pg.tile([128, KK_H * VOCAB], BF16, tag="linWT")
            lb_sb = pg.tile([128, VOCAB], F32, tag="lb")
            # h ring: [p, k*(4*BL) + slot*BL + b] so a step-pair's k-chunk
            # slice is contiguous (matmul lhsT needs a single free dim)
            H4 = pg.tile([128, 4 * HB], BF16, tag="H4")

            # whh load order follows the gate stream order, split in 8 so
            # small tokp writes interleave on the DMA rings
            whh_chunks = []
            for gi in GSTREAM:
                c0 = gi * 8 * KK_H * 128
                cm = c0 + 4 * KK_H * 128
                c1 = c0 + 8 * KK_H * 128
                whh_chunks.append((c0, cm))
                whh_chunks.append((cm, c1))

            for m in range(NVT):
                mrows = min(128, VOCAB - 128 * m)
                for nb in range(G4 // 512):
                    ps = pap.tile([128, 512], F32, tag="tp_ps", bufs=8)
                    for k in range(KK_E):
                        nc.tensor.matmul(
                            ps[:mrows],
                            lhsT=embWT_sb[:, k * VOCAB + 128 * m:
                                          k * VOCAB + 128 * m + mrows],
                            rhs=WihT_sb[:, k * G4 + 512 * nb:
                                        k * G4 + 512 * (nb + 1)],
                            start=(k == 0), stop=(k == KK_E - 1),
                        )
                    sb = pa.tile([128, 512], BF16, tag="tp_sb", bufs=24)
                    nc.vector.tensor_add(
                        sb[:mrows], ps[:mrows],
                        bias_sb[:mrows, 512 * nb:512 * (nb + 1)])
                    nc.sync.dma_start(
                        tokp[128 * m:128 * m + mrows, 512 * nb:512 * (nb + 1)],
                        sb[:mrows])
                c0, c1 = whh_chunks[m]
                nc.sync.dma_start(whh_sb[:, c0:c1], whhT[:, c0:c1])
                if m == NVT - 1:
                    nc.sync.dma_start(featT_sb[:], featT)
                    nc.sync.dma_start(caps_sb[:], caps)

        # ================= recurrence + in-loop logits ===================
        with (
            tc.tile_pool(name="pb", bufs=1) as pb,
            tc.tile_pool(name="pbp", bufs=1, space="PSUM") as pbp,
        ):
            H4v = H4[:].rearrange("p (k s b) -> p k s b", k=KK_H, s=4)

            # deferred constant loads (needed from t>=2 only)
            nc.sync.dma_start(linWT_sb[:], linWT)
            nc.sync.dma_start(lb_sb[:], linb.to_broadcast((128, VOCAB)))

            c_cur = pb.tile([128, HB], F32, tag="c", bufs=2, name="c_init")
            nc.vector.memset(c_cur[:], 0.0)

            xp_t = {}
            xpT_t = {}

            def gather(t):
                xp_t[t] = pb.tile([BL, G4], BF16, tag="xp", bufs=3,
                                  name=f"xp_{t}")
                nc.gpsimd.indirect_dma_start(
                    out=xp_t[t][:], out_offset=None, in_=tokp,
                    in_offset=bass.IndirectOffsetOnAxis(
                        ap=caps_sb[:, t:t + 1], axis=0),
                )

            def transp(t):
                xpT_t[t] = pb.tile([128, NGJ * BL], BF16, tag="xpT", bufs=2,
                                   name=f"xpT_{t}")
                nc.sync.dma_start(
                    xpT_t[t][:].rearrange("p (j b) -> p j b", j=NGJ),
                    xp_t[t][:], transpose=True)
                del xp_t[t]

            gather(0)
            gather(1)
            transp(0)

            lp_cur = [None]

            def logits_half(p, nh):
                # logits for out steps 2p, 2p+1 from h_{2p+1}, h_{2p+2}
                # (H4 slots 2p%4, 2p%4+1 -- never wraps since 2p%4 in {0,2})
                if nh == 0:
                    lp_cur[0] = pbp.tile([128, 1024], F32, tag="lp", bufs=2,
                                         name=f"lp_{p}")
                lp = lp_cur[0]
                n0, n1 = (0, 512) if nh == 0 else (512, VOCAB)
                s0 = (2 * p) % 4
                for k in range(KK_H):
                    nc.tensor.matmul(
                        lp[:, n0:n1],
                        lhsT=H4[:, k * 4 * BL + s0 * BL:
                                k * 4 * BL + (s0 + 2) * BL],
                        rhs=linWT_sb[:, k * VOCAB + n0:k * VOCAB + n1],
                        start=(k == 0), stop=(k == KK_H - 1))

            def logits_tail(p):
                ls = pb.tile([128, VOCAB], F32, tag="ls", bufs=2,
                             name=f"ls_{p}")
                nc.vector.tensor_add(ls[:], lp_cur[0][:, 0:VOCAB], lb_sb[:])
                nc.sync.dma_start(out[:, 2 * p, :], ls[0:BL])
                nc.sync.dma_start(out[:, 2 * p + 1, :], ls[BL:128])

            for t in range(steps):
                if t + 2 < steps:
                    gather(t + 2)
                if t + 1 < steps:
                    transp(t + 1)

                if t == 0:
                    def hs(k):
                        return featT_sb[:, k * BL:(k + 1) * BL]
                else:
                    sp = (t - 1) % 4

                    def hs(k, sp=sp):
                        return H4[:, k * 4 * BL + sp * BL:
                                  k * 4 * BL + (sp + 1) * BL]

                # gate PSUM init: xpT enters via identity matmul (cheap on
                # PE, keeps the DVE queue clear and ACT reads PSUM directly)
                gps = {}
                for gi in GSTREAM:
                    gps[gi] = pbp.tile([128, 512], F32, tag="gps", bufs=4,
                                       name=f"g{gi}_{t}")
                    nc.tensor.matmul(
                        gps[gi][:], lhsT=ident[:],
                        rhs=xpT_t[t][:, gi * 512:(gi + 1) * 512],
                        start=True, stop=False)
                del xpT_t[t]

                # PE gap filler: logits half-pair (reads h from >=2 steps
                # back, so it runs while the current h-tail is in flight)
                if t >= 3 and t % 2 == 1:
                    logits_half((t - 3) // 2, 0)
                elif t >= 4 and t % 2 == 0:
                    logits_half((t - 4) // 2, 1)

                # recurrent gate matmuls: W_hh tiles stationary, hT moving
                for gi in GSTREAM:
                    base = gi * 8 * KK_H * 128
                    for j in range(8):
                        oap = gps[gi][:, j * BL:(j + 1) * BL]
                        for k in range(KK_H):
                            nc.tensor.matmul(
                                oap,
                                lhsT=whh_sb[:, base + (j * KK_H + k) * 128:
                                            base + (j * KK_H + k + 1) * 128],
                                rhs=hs(k),
                                start=False, stop=(j == 7 and k == KK_H - 1),
                            )

                # activations straight from PSUM
                ag = pb.tile([128, HB], BF16, tag="ag", name=f"ag_{t}")
                nc.scalar.activation(ag[:], gps[2][:], AF.Tanh)
                ai = pb.tile([128, HB], BF16, tag="ai", name=f"ai_{t}")
                nc.scalar.activation(ai[:], gps[0][:], AF.Sigmoid)
                af = pb.tile([128, HB], BF16, tag="af", name=f"af_{t}")
                nc.scalar.activation(af[:], gps[1][:], AF.Sigmoid)

                t2 = pb.tile([128, HB], F32, tag="t2", name=f"t2_{t}")
                nc.vector.tensor_mul(t2[:], ai[:], ag[:])
                t1 = pb.tile([128, HB], F32, tag="t1", name=f"t1_{t}")
                nc.vector.tensor_mul(t1[:], af[:], c_cur[:])
                c_new = pb.tile([128, HB], F32, tag="c", bufs=2,
                                name=f"c_{t}")
                nc.vector.tensor_add(c_new[:], t1[:], t2[:])
                tcb = pb.tile([128, HB], BF16, tag="tc", name=f"tc_{t}")
                nc.scalar.activation(tcb[:], c_new[:], AF.Tanh)

                # o-gate tail in halves so the first hT chunks land early
                sw = t % 4
                for hh in range(2):
                    sl = slice(hh * 256, (hh + 1) * 256)
                    ao = pb.tile([128, 256], BF16, tag="ao", bufs=2,
                                 name=f"ao{hh}_{t}")
                    nc.scalar.activation(ao[:], gps[3][:, sl], AF.Sigmoid)
                    t3 = pb.tile([128, 256], BF16, tag="t3", bufs=2,
                                 name=f"t3{hh}_{t}")
                    nc.vector.tensor_mul(t3[:], ao[:], tcb[:, sl])
                    nc.vector.tensor_add(
                        H4v[:, hh * 4:(hh + 1) * 4, sw, :],
                        t3[:], featT_sb[:, sl])
                c_cur = c_new

                # logits epilogue late in the step (keeps DVE queue clear)
                if t >= 4 and t % 2 == 0:
                    logits_tail((t - 4) // 2)
                elif t == steps - 1:
                    # pull p=30's second half into the last step
                    logits_half((steps - 4) // 2, 1)

            # trailing logits
            logits_tail((steps - 4) // 2)
            logits_half((steps - 2) // 2, 0)      # p=31
            logits_half((steps - 2) // 2, 1)
            logits_tail((steps - 2) // 2)

# ---------------------------------------------------------------------------
# host glue
# ---------------------------------------------------------------------------

_CACHE = {}


def _get_program(steps=TS):
    if steps not in _CACHE:
        _CACHE[steps] = build_program(steps)
    return _CACHE[steps]


def chunkT(w, kk):
    # [R, C] -> [128, kk*C]: [p, k*C + c] = w.T[k*128+p, c] = w[c, k*128+p]
    f32 = np.float32
    wt = np.ascontiguousarray(w.T.astype(f32))
    r = wt.reshape(kk, 128, w.shape[0])
    return np.ascontiguousarray(r.transpose(1, 0, 2).reshape(128, -1))


def make_in_maps(feature, captions, embed_W, W_ih, W_hh, b_ih, b_hh,
                 lin_W, lin_b):
    f32 = np.float32
    bf16 = ml_dtypes.bfloat16

    embWT_p = chunkT(embed_W, KK_E)          # [128, 4*1004]
    WihT_p = chunkT(W_ih, KK_E)              # [128, 4*4096]
    blob_a = np.concatenate([embWT_p, WihT_p], axis=1).astype(bf16)

    # whhT: [p, (GJ*8 + k)*128 + m] = W_hh[GJ*128 + m, k*128 + p]
    arr = W_hh.astype(f32).reshape(NGJ, 128, KK_H, 128)   # [GJ, m, k, p]
    whhT_p = np.ascontiguousarray(
        arr.transpose(3, 0, 2, 1).reshape(128, NGJ * KK_H * 128)).astype(bf16)

    linWT_p = chunkT(lin_W, KK_H).astype(bf16)   # [128, 8*1004]

    shared = {
        "blob_a": np.ascontiguousarray(blob_a),
        "biasg": (b_ih + b_hh).astype(f32).reshape(1, G4).astype(bf16),
        "whhT": whhT_p,
        "linWT": np.ascontiguousarray(linWT_p),
        "linb": lin_b.astype(f32).reshape(1, VOCAB),
    }
    in_maps = []
    for i in range(NCORES):
        sl = slice(i * BL, (i + 1) * BL)
        fl = np.ascontiguousarray(feature[sl].astype(f32))
        featT_p = np.ascontiguousarray(
            fl.T.reshape(KK_H, 128, BL).transpose(1, 0, 2).reshape(128, HB))
        m = dict(shared)
        m["featT"] = featT_p.astype(bf16)
        m["caps"] = np.ascontiguousarray(captions[sl, :TS].astype(np.int32))
        in_maps.append(m)
    return in_maps


def kernel(feature, captions, lengths=None, embed_W=None, W_ih=None,
           W_hh=None, b_ih=None, b_hh=None, lin_W=None, lin_b=None,
           trace=False):
    feature = np.asarray(feature)
    captions = np.asarray(captions)
    nc = _get_program()
    in_maps = make_in_maps(
        feature, captions, np.asarray(embed_W), np.asarray(W_ih),
        np.asarray(W_hh), np.asarray(b_ih), np.asarray(b_hh),
        np.asarray(lin_W), np.asarray(lin_b))
    res = run_bass_kernel_spmd(nc, in_maps, list(range(NCORES)), trace=trace)
    outp = np.concatenate([res.results[i]["out"] for i in range(NCORES)], axis=0)
    if trace:
        kernel.last_exec_time_ns = res.exec_time_ns
        kernel.last_results = res
    return outp
